# revision 21
# baseline (speedup 1.0000x reference)
"""ChebNet (K=4, 2 ChebConv layers + relu + log_softmax) on 8 trn2 NeuronCores.

Strategy (graph/data parallel, dense-ified SpMM + host-precomputed Chebyshev
polynomial matrices):
  - prop matrices act on g = fp8(dis * value):
      T1 = A_hat x        = -diag(dis) @ Cnt^T @ g          (Cnt exact in fp8)
      T2 = (2A^2 - I) x   = desc2 * (M2 @ g) - x,   M2 = 2A^2 diag(1/dis)
      T3 = (4A^3 - 3A) x  = desc3 * (M3 @ g) - 3*T1, M3 = 4A^3 diag(1/dis)
    M2/M3 are host-precomputed, row-scaled (pow2) and fp8-quantized; the
    -x / -3*T1 corrections happen on-device in fp32, which keeps the
    quantization error at the same level as the pure-recursion design
    (validated 4.4e-3 vs reference).
  - Nodes sharded 8 ways by destination. Cnt^T shard (12.6 MB fp8) stays
    SBUF-resident; M2/M3 shards are STREAMED from HBM through a small
    rotating window (each entry is used once per layer), so the kernel
    needs only ONE collective in total: the AllGather of h between the
    layers. This sidesteps the collective path's variable 60-140us cold
    start and its ~10us/chunk service pace that dominated the
    recursion-based design (5 exchanges, 15 collectives, ~320us).
  - All matmuls are fp8 DoubleRow (256-src contraction / instruction,
    157 TF/s). Layer-1 T1 is emitted in A-arrival order so the PE streams
    right behind the initial HBM load.
  - Tail: biases/casts on DVE, log_softmax batched with the Exp ACT table
    primed early (ACT_TABLE_LOAD is a 1.28us stall), single output DMA.
"""

import sys

sys.path.insert(0, "/opt/trn_rl_repo")

import numpy as np
import ml_dtypes

import concourse.bacc as bacc
import concourse.mybir as mybir
import concourse.tile as tile
from concourse.bass_utils import run_bass_kernel_spmd
from concourse.masks import make_identity

F32 = mybir.dt.float32
BF16 = mybir.dt.bfloat16
F8E4 = mybir.dt.float8e4

NCORES = 8
P = 128

N = 10000
F_IN = 128
HID = 128
C_OUT = 16
K_ORD = 4

N_STREAM_SLOTS = 2


class Geom:
    def __init__(self, n_nodes, tiles_per_core, f_in=F_IN, hid=HID, c_out=C_OUT,
                 k_ord=K_ORD):
        self.n = n_nodes
        self.tpc = tiles_per_core          # src tiles per core (DLOC/128)
        self.dloc = tiles_per_core * P     # nodes per core (padded)
        self.npad = self.dloc * NCORES     # padded node count
        self.nt = self.npad // P           # total src tiles
        self.f = f_in
        self.hid = hid
        self.c = c_out
        self.k = k_ord
        assert self.npad >= n_nodes
        assert f_in == P and hid == P
        # psum chunking of the dloc free dim (max 512 fp32 per bank).
        self.chunks = []
        off = 0
        while off < self.dloc:
            sz = 256 if (off == 0 and self.dloc > 512) else \
                min(512, self.dloc - off)
            self.chunks.append((off, sz))
            off += sz
        self.ctiles = [(off // P, (off + sz) // P) for off, sz in self.chunks]
        assert all((t1 - t0) % 2 == 0 for t0, t1 in self.ctiles)
        # a-tile groups (DMA granularity, also the stream piece size).
        # 16 tiles -> 8KB per-partition DMA rows; smaller pieces fall off
        # the descriptor-efficiency cliff (~2KB rows ran at ~215GB/s).
        self.ag = next(a for a in (16, 8, 40) if self.nt % a == 0)
        self.n_agrp = self.nt // self.ag
        self.n_ggrp = NCORES

    def gci_of_tile(self, t):
        for gci, (t0, t1) in enumerate(self.ctiles):
            if t0 <= t < t1:
                return gci
        raise AssertionError(t)


FULL = Geom(N, 10)  # 1280 nodes/core, npad=10240, 80 src tiles


def build_nc(g: Geom):
    nc = bacc.Bacc("TRN2", target_bir_lowering=False, debug=False,
                   num_devices=NCORES)

    # ---- kernel I/O ----------------------------------------------------
    a_in = [nc.dram_tensor(f"a_in_c{ci}", [g.n_agrp, P, g.ag, sz], F8E4,
                           kind="ExternalInput")
            for ci, (off, sz) in enumerate(g.chunks)]
    m2_in = [nc.dram_tensor(f"m2_in_c{ci}", [g.n_agrp, P, g.ag, sz], F8E4,
                            kind="ExternalInput")
             for ci, (off, sz) in enumerate(g.chunks)]
    m3_in = [nc.dram_tensor(f"m3_in_c{ci}", [g.n_agrp, P, g.ag, sz], F8E4,
                            kind="ExternalInput")
             for ci, (off, sz) in enumerate(g.chunks)]
    g0_in = nc.dram_tensor("g0_in", [P, g.nt, g.f], F8E4, kind="ExternalInput")
    xt_in = nc.dram_tensor("xt_in", [P, g.dloc], F32, kind="ExternalInput")
    disp_in = nc.dram_tensor("disp_in", [P, g.dloc], F32, kind="ExternalInput")
    desc2_in = nc.dram_tensor("desc2_in", [P, g.dloc], F32,
                              kind="ExternalInput")
    desc3_in = nc.dram_tensor("desc3_in", [P, g.dloc], F32,
                              kind="ExternalInput")
    dispt_in = nc.dram_tensor("dispt_in", [P, g.tpc], F32,
                              kind="ExternalInput")
    w1_in = nc.dram_tensor("w1_in", [P, g.k, g.hid], BF16,
                           kind="ExternalInput")
    w2_in = nc.dram_tensor("w2_in", [P, g.k, g.c], BF16, kind="ExternalInput")
    bb_in = nc.dram_tensor("bb_in", [P, 2], F32, kind="ExternalInput")

    out_dram = nc.dram_tensor("out", [g.dloc, g.c], F32, kind="ExternalOutput")

    groups = [list(range(NCORES))]
    S = N_STREAM_SLOTS

    with tile.TileContext(nc) as tc:
        with (
            tc.tile_pool(name="pers", bufs=1) as pers,
            tc.tile_pool(name="work", bufs=1) as work,
            tc.tile_pool(name="psum", bufs=1, space="PSUM") as psp,
            tc.tile_pool(name="dram", bufs=1, space="DRAM") as drp,
        ):
            # ---- persistent SBUF ---------------------------------------
            a_sb = [[pers.tile([P, g.ag, sz], F8E4, tag=f"a{ci}_{i}",
                                name=f"a{ci}_{i}")
                     for i in range(g.n_agrp)]
                    for ci, (off, sz) in enumerate(g.chunks)]
            # single g buffer set: g0 for layer 1, h (AG result) for layer 2
            gbufC = [pers.tile([P, g.n_ggrp, t1 - t0, g.f], F8E4,
                               tag=f"g_{ci}", name=f"g_{ci}")
                     for ci, (t0, t1) in enumerate(g.ctiles)]
            t_sb = [pers.tile([P, g.dloc], F32, tag=f"t{i}", name=f"t{i}")
                    for i in range(3)]
            disp = pers.tile([P, g.dloc], F32, name="disp")
            desc2 = pers.tile([P, g.dloc], F32, name="desc2")
            desc3 = pers.tile([P, g.dloc], F32, name="desc3")
            dispt = pers.tile([P, g.tpc], F32, name="dispt")
            acc = pers.tile([P, g.dloc], F32, name="acc")
            tbf = pers.tile([P, g.dloc], BF16, name="tbf")
            tb0 = pers.tile([P, g.dloc], BF16, name="tb0")
            tstage = pers.tile([P, g.tpc, g.f], F8E4, name="tstage")
            w1_sb = pers.tile([P, g.k, g.hid], BF16, name="w1_sb")
            w2_sb = pers.tile([P, g.k, g.c], BF16, name="w2_sb")
            bb_sb = pers.tile([P, 2], F32, name="bb_sb")
            idf32 = pers.tile([P, P], F32, name="idf32")
            t_sb0 = pers.tile([P, g.dloc], F32, name="xt")
            prime = work.tile([P, 1], F32, name="prime")

            # ---- DRAM bounce buffers for the single h exchange ---------
            ag_src = drp.tile([P, g.tpc * g.f], F8E4, name="ag_src")
            ag_dst = drp.tile([NCORES * P, g.tpc * g.f], F8E4,
                              addr_space="Shared", name="ag_dst")
            make_identity(nc, idf32[:])

            # ---- initial loads: g0 + A chunk0 race first, then smalls,
            # then A chunks 1,2. bulk on scalar+gpsimd; sync reserved for
            # the latency-critical stage/g-load/out path.
            bulk = [nc.scalar, nc.gpsimd]
            qctr = {"i": 0}

            def bulk_load(dst, src):
                bulk[qctr["i"] % 2].dma_start(dst, src)
                qctr["i"] += 1

            g0_4d = g0_in.ap().rearrange("p (j t) f -> p j t f", j=g.n_ggrp)
            for ci, (t0, t1) in enumerate(g.ctiles):
                bulk_load(gbufC[ci][:], g0_4d[:, :, t0:t1, :])
            for i in range(g.n_agrp):
                bulk_load(a_sb[0][i][:], a_in[0][i])
            smalls = [(t_sb0, xt_in), (w1_sb, w1_in), (w2_sb, w2_in),
                      (bb_sb, bb_in), (dispt, dispt_in), (disp, disp_in),
                      (desc2, desc2_in), (desc3, desc3_in)]
            for (dst, src) in smalls:
                bulk_load(dst[:], src.ap())
            for ci in range(1, len(g.chunks)):
                for i in range(g.n_agrp):
                    bulk_load(a_sb[ci][i][:], a_in[ci][i])

            # rotating stream windows for M2/M3 pieces
            ms_sb = [[pers.tile([P, g.ag, sz], F8E4, tag=f"ms{ci}_{s}",
                                name=f"ms{ci}_{s}")
                      for s in range(S)]
                     for ci, (off, sz) in enumerate(g.chunks)]
            stream_ctr = {"i": 0}

            n_pairs = g.nt // 2  # per output chunk

            def pairs_arrival():
                """ascending gi == src-tile arrival order."""
                return [(j * g.tpc + t)
                        for j in range(g.n_ggrp)
                        for t in range(0, g.tpc, 2)]

            def pairs_gci():
                """AG-chunk arrival order (for layer-2 T1)."""
                return [(j * g.tpc + g.ctiles[gci][0] + 2 * p)
                        for gci in range(len(g.ctiles))
                        for j in range(g.n_ggrp)
                        for p in range((g.ctiles[gci][1]
                                        - g.ctiles[gci][0]) // 2)]

            def lhs_of(gi):
                j, t = gi // g.tpc, gi % g.tpc
                gci = g.gci_of_tile(t)
                ts0 = g.ctiles[gci][0]
                return gbufC[gci][:, j, t - ts0:t - ts0 + 2, :]

            def emit_resident(pp, ci, sz, gi_list):
                """DoubleRow matmuls vs the resident Cnt shard."""
                for n_i, gi in enumerate(gi_list):
                    rhs = a_sb[ci][gi // g.ag][:, gi % g.ag:gi % g.ag + 2, :]
                    nc.tensor.matmul(
                        pp[:, :sz], lhsT=lhs_of(gi), rhs=rhs,
                        start=(n_i == 0), stop=(n_i == n_pairs - 1),
                        perf_mode=mybir.MatmulPerfMode.DoubleRow,
                    )

            def emit_streamed(pp, ci, sz, m_in, into_a=False):
                """DoubleRow matmuls vs streamed pieces of M2/M3. Layer-2
                re-streams overwrite the dead Cnt shard (into_a) for
                unbounded lookahead."""
                for agrp in range(g.n_agrp):
                    if into_a:
                        slot = a_sb[ci][agrp]
                    else:
                        slot = ms_sb[ci][stream_ctr["i"] % S]
                        stream_ctr["i"] += 1
                    bulk_load(slot[:], m_in[ci][agrp])
                    for p_i in range(g.ag // 2):
                        gi = agrp * g.ag + 2 * p_i
                        n_i = agrp * (g.ag // 2) + p_i
                        nc.tensor.matmul(
                            pp[:, :sz], lhsT=lhs_of(gi),
                            rhs=slot[:, 2 * p_i:2 * p_i + 2, :],
                            start=(n_i == 0), stop=(n_i == n_pairs - 1),
                            perf_mode=mybir.MatmulPerfMode.DoubleRow,
                        )

            def w_term_chunk(w_sb, k, rhs_bf, cdim, off, sz):
                """acc[0:cdim, chunk] (+)= (T_k @ W[k])^T, bf16 matmul."""
                wt = psp.tile([P, 512], F32, space="PSUM", tag="wt",
                              name="wt", bufs=2)
                nc.tensor.matmul(
                    wt[:cdim, :sz],
                    lhsT=w_sb[:, k, :],
                    rhs=rhs_bf[:, off:off + sz],
                    start=True, stop=True,
                )
                if k == 0:
                    nc.vector.tensor_copy(acc[:cdim, off:off + sz],
                                          wt[:cdim, :sz])
                else:
                    nc.vector.tensor_add(acc[:cdim, off:off + sz],
                                         acc[:cdim, off:off + sz],
                                         wt[:cdim, :sz])

            def chunk_tiles(off, sz):
                return range(off // P, (off + sz) // P)

            def stage_chunk(ci, src_f32, off, sz):
                """transpose fp32 tiles on the PE, fuse dis-scale + fp8 cast
                in the PSUM->SBUF copy, stage to ag_src."""
                for t in chunk_tiles(off, sz):
                    tpb = psp.tile([P, P], F32, space="PSUM", tag="tpb",
                                   name="tpb", bufs=2)
                    nc.tensor.transpose(out=tpb[:],
                                        in_=src_f32[:, t * P:(t + 1) * P],
                                        identity=idf32[:])
                    nc.scalar.mul(tstage[:, t, :], tpb[:], dispt[:, t:t + 1])
                t0, t1 = off // P, (off + sz) // P
                nc.sync.dma_start(ag_src[:, t0 * g.f:t1 * g.f],
                                  tstage[:, t0:t1, :])

            def allgather_h():
                # ONE collective for the whole h shard: each extra
                # collective costs ~17us of CC re-arm on top of the wait.
                nc.gpsimd.collective_compute(
                    "AllGather",
                    mybir.AluOpType.bypass,
                    replica_groups=groups,
                    ins=[ag_src[:]],
                    outs=[ag_dst[:]],
                )
                # gbufC held g0 until ALL layer-1 reads finished (every T3
                # output chunk contracts over every gci), so these loads
                # come after the whole T3 phase.
                ag4d = ag_dst[:, :].rearrange("(j p) (t f) -> p j t f",
                                              p=P, f=g.f)
                for cj, (t0, t1) in enumerate(g.ctiles):
                    nc.sync.dma_start(gbufC[cj][:], ag4d[:, :, t0:t1, :])

            z_all = work.tile([P, g.tpc, g.c], F32, name="z_all")
            m_all = work.tile([P, g.tpc, 1], F32, name="m_all")
            e_all = work.tile([P, g.tpc, g.c], F32, name="e_all")
            s_all = work.tile([P, g.tpc, 1], F32, name="s_all")
            o_all = work.tile([P, g.tpc, g.c], F32, name="o_all")
            out_ap = out_dram.ap().rearrange("(t p) c -> p t c", p=P)

            def final_chunk_tail(ci, off, sz):
                t0, t1 = off // P, (off + sz) // P
                nt = t1 - t0
                nc.vector.tensor_tensor(
                    out=acc[:g.c, off:off + sz],
                    in0=acc[:g.c, off:off + sz],
                    in1=bb_sb[:g.c, 1:2].to_broadcast([g.c, sz]),
                    op=mybir.AluOpType.add)
                for t in chunk_tiles(off, sz):
                    zp = psp.tile([P, g.c], F32, space="PSUM",
                                  tag="tpb", name="zp", bufs=2)
                    nc.tensor.transpose(
                        out=zp[:],
                        in_=acc[:g.c, t * P:(t + 1) * P],
                        identity=idf32[:g.c, :g.c])
                    nc.vector.tensor_copy(z_all[:, t, :], zp[:])
                z = z_all[:, t0:t1, :]
                m = m_all[:, t0:t1, :]
                nc.vector.tensor_reduce(out=m[:, :, 0], in_=z,
                                        axis=mybir.AxisListType.X,
                                        op=mybir.AluOpType.max)
                nc.vector.tensor_tensor(out=e_all[:, t0:t1, :], in0=z,
                                        in1=m.to_broadcast([P, nt, g.c]),
                                        op=mybir.AluOpType.subtract)

            def final_softmax_tail():
                nc.scalar.activation(o_all[:], e_all[:],
                                     mybir.ActivationFunctionType.Exp)
                nc.vector.tensor_reduce(out=s_all[:, :, 0], in_=o_all[:],
                                        axis=mybir.AxisListType.X,
                                        op=mybir.AluOpType.add)
                nc.scalar.activation(s_all[:], s_all[:],
                                     mybir.ActivationFunctionType.Ln)
                nc.vector.tensor_tensor(
                    out=o_all[:], in0=e_all[:],
                    in1=s_all[:].to_broadcast([P, g.tpc, g.c]),
                    op=mybir.AluOpType.subtract)
                nc.sync.dma_start(out_ap[:, :, :], o_all[:])

            def stt(out_t, sl, pp, sz, scalar, in1_t):
                nc.vector.scalar_tensor_tensor(
                    out=out_t[:, sl], in0=pp[:, :sz], scalar=scalar,
                    in1=in1_t[:, sl],
                    op0=mybir.AluOpType.mult, op1=mybir.AluOpType.mult)

            # ---- the two ChebConv layers -------------------------------
            # t_sb roles: [0] = U scratch then h, [1] = T1, [2] = T2 then T3
            for layer in range(2):
                w_sb = w1_sb if layer == 0 else w2_sb
                cdim = g.hid if layer == 0 else g.c
                last_ci = len(g.chunks) - 1

                # T0 W-term (fills the A-load / h-AG wait)
                if layer == 0:
                    nc.vector.tensor_copy(tb0[:], t_sb0[:])
                for (off, sz) in g.chunks:
                    w_term_chunk(w_sb, 0, tb0, cdim, off, sz)

                # ---- T1 phase (resident Cnt) ---------------------------
                order = pairs_arrival() if layer == 0 else pairs_gci()
                for ci, (off, sz) in enumerate(g.chunks):
                    sl = slice(off, off + sz)
                    pp = psp.tile([P, 512], F32, space="PSUM", tag="pp",
                                  name=f"ppt1_{ci}", bufs=3)
                    emit_resident(pp, ci, sz, order)
                    stt(t_sb[1], sl, pp, sz, -1.0, disp)
                    nc.vector.tensor_copy(tbf[:, sl], t_sb[1][:, sl])
                    w_term_chunk(w_sb, 1, tbf, cdim, off, sz)

                # ---- T2 phase (streamed M2): T2 = desc2*pp - T0 --------
                t0_t = t_sb0 if layer == 0 else t_sb[0]
                for ci, (off, sz) in enumerate(g.chunks):
                    sl = slice(off, off + sz)
                    pp = psp.tile([P, 512], F32, space="PSUM", tag="pp",
                                  name=f"ppt2_{ci}", bufs=3)
                    emit_streamed(pp, ci, sz, m2_in, into_a=(layer == 1))
                    stt(t_sb[2], sl, pp, sz, 1.0, desc2)
                    nc.vector.tensor_sub(t_sb[2][:, sl], t_sb[2][:, sl],
                                         t0_t[:, sl])
                    nc.vector.tensor_copy(tbf[:, sl], t_sb[2][:, sl])
                    w_term_chunk(w_sb, 2, tbf, cdim, off, sz)

                # ---- T3 phase (streamed M3): T3 = desc3*pp - 3*T1 ------
                for ci, (off, sz) in enumerate(g.chunks):
                    sl = slice(off, off + sz)
                    pp = psp.tile([P, 512], F32, space="PSUM", tag="pp",
                                  name=f"ppt3_{ci}", bufs=3)
                    emit_streamed(pp, ci, sz, m3_in, into_a=(layer == 1))
                    stt(t_sb[0], sl, pp, sz, 1.0, desc3)   # U
                    nc.vector.scalar_tensor_tensor(
                        out=t_sb[2][:, sl], in0=t_sb[1][:, sl], scalar=-3.0,
                        in1=t_sb[0][:, sl],
                        op0=mybir.AluOpType.mult, op1=mybir.AluOpType.add)
                    nc.vector.tensor_copy(tbf[:, sl], t_sb[2][:, sl])
                    w_term_chunk(w_sb, 3, tbf, cdim, off, sz)
                    if layer == 0:
                        # h = relu(acc + b1); stage fp8(dis*h); exchange
                        nc.scalar.activation(
                            t_sb[0][:, sl], acc[:, sl],
                            mybir.ActivationFunctionType.Relu,
                            bias=bb_sb[:, 0:1], scale=1.0)
                        stage_chunk(ci, t_sb[0], off, sz)
                        nc.vector.tensor_copy(tb0[:, sl], t_sb[0][:, sl])
                        if ci == last_ci:
                            # prime the Exp ACT table during layer-2 slack
                            nc.scalar.activation(
                                prime[:], bb_sb[:, 0:1],
                                mybir.ActivationFunctionType.Exp)
                            allgather_h()
                    else:
                        final_chunk_tail(ci, off, sz)
                        if ci == last_ci:
                            final_softmax_tail()

    nc.compile()
    return nc


def host_prep(g: Geom, x, edge_index, W1, b1, W2, b2):
    """Shard + dense-ify + precompute the Chebyshev polynomial matrices."""
    import scipy.sparse as sp
    n = g.n
    src = np.asarray(edge_index[0], dtype=np.int64)
    dst = np.asarray(edge_index[1], dtype=np.int64)
    deg = np.bincount(src, minlength=n).astype(np.float64)
    dis = np.where(deg > 0, 1.0 / np.sqrt(np.maximum(deg, 1e-12)), 0.0)

    dis_pad = np.zeros(g.npad, dtype=np.float64)
    dis_pad[:n] = dis
    x_pad = np.zeros((g.npad, g.f), dtype=np.float32)
    x_pad[:n] = np.asarray(x, dtype=np.float32)

    w = np.ones(src.shape[0])
    Csp = sp.csr_matrix((w, (dst, src)), shape=(g.npad, g.npad))
    Ah = (sp.diags(-dis_pad) @ Csp @ sp.diags(dis_pad)).tocsr()
    A2 = (Ah @ Ah).toarray().astype(np.float32)
    A3 = (Ah @ A2).astype(np.float32)

    inv_dis = np.where(dis_pad > 0, 1.0 / np.maximum(dis_pad, 1e-12), 0.0
                       ).astype(np.float32)
    M2 = (2.0 * A2) * inv_dis[None, :]
    M3 = (4.0 * A3) * inv_dis[None, :]
    del A2, A3

    def rowquant(M, target=200.0):
        mx = np.abs(M).max(1)
        mx = np.maximum(mx, 1e-30)
        s = np.exp2(np.floor(np.log2(target / mx))).astype(np.float32)
        Mq = (M * s[:, None]).astype(ml_dtypes.float8_e4m3)
        desc = (1.0 / s).astype(np.float32)
        return Mq, desc

    M2q, d2 = rowquant(M2)
    del M2
    M3q, d3 = rowquant(M3)
    del M3

    # dense-ified edge-count matrix, transposed: cnt_t[s, d]
    cnt_t = np.zeros((g.npad, g.npad), dtype=np.float32)
    np.add.at(cnt_t, (src, dst), 1.0)
    cnt_q = cnt_t.astype(ml_dtypes.float8_e4m3)
    del cnt_t

    g0 = (dis_pad[:, None] * x_pad).astype(np.float32)
    g0_tiles = (g0.reshape(g.nt, P, g.f).transpose(1, 0, 2)
                .astype(ml_dtypes.float8_e4m3))  # [128, nt, f]

    w1 = np.ascontiguousarray(
        np.asarray(W1, np.float32).transpose(1, 0, 2)
    ).astype(ml_dtypes.bfloat16)  # [P, k, hid]
    w2 = np.ascontiguousarray(
        np.asarray(W2, np.float32).transpose(1, 0, 2)
    ).astype(ml_dtypes.bfloat16)  # [P, k, c]
    bb = np.zeros((P, 2), np.float32)
    bb[:g.hid, 0] = np.asarray(b1, np.float32)
    bb[:g.c, 1] = np.asarray(b2, np.float32)

    def shard_mat(Mq_srcdst, lo, hi):
        """[src, dst] fp8 -> per-chunk [n_agrp, P, ag, sz]."""
        mc = (Mq_srcdst[:, lo:hi]
              .reshape(g.n_agrp, g.ag, P, g.dloc).transpose(0, 2, 1, 3))
        return [np.ascontiguousarray(mc[:, :, :, off:off + sz])
                for (off, sz) in g.chunks]

    # M2q/M3q are [dst, src]; device wants [src, dst_local]
    M2qT = np.ascontiguousarray(M2q.T)
    M3qT = np.ascontiguousarray(M3q.T)

    in_maps = []
    for c in range(NCORES):
        lo, hi = c * g.dloc, (c + 1) * g.dloc
        a_chunks = shard_mat(cnt_q, lo, hi)
        m2_chunks = shard_mat(M2qT, lo, hi)
        m3_chunks = shard_mat(M3qT, lo, hi)
        xt = np.ascontiguousarray(x_pad[lo:hi].T)          # [128, dloc]
        d_loc = dis_pad[lo:hi].astype(np.float32)
        disp = np.ascontiguousarray(
            np.broadcast_to(d_loc[None, :], (P, g.dloc))).astype(np.float32)
        desc2b = np.ascontiguousarray(np.broadcast_to(
            d2[lo:hi][None, :], (P, g.dloc))).astype(np.float32)
        desc3b = np.ascontiguousarray(np.broadcast_to(
            d3[lo:hi][None, :], (P, g.dloc))).astype(np.float32)
        dispt = np.ascontiguousarray(
            d_loc.reshape(g.tpc, P).T).astype(np.float32)  # [128, tpc]
        im = {f"a_in_c{ci}": a_chunks[ci] for ci in range(len(g.chunks))}
        im.update({f"m2_in_c{ci}": m2_chunks[ci]
                   for ci in range(len(g.chunks))})
        im.update({f"m3_in_c{ci}": m3_chunks[ci]
                   for ci in range(len(g.chunks))})
        im.update({
            "g0_in": np.ascontiguousarray(g0_tiles),
            "xt_in": xt,
            "disp_in": disp,
            "desc2_in": desc2b,
            "desc3_in": desc3b,
            "dispt_in": dispt,
            "w1_in": w1,
            "w2_in": w2,
            "bb_in": bb,
        })
        in_maps.append(im)
    return in_maps


_CACHED_NC = None


def _get_nc():
    global _CACHED_NC
    if _CACHED_NC is None:
        _CACHED_NC = build_nc(FULL)
    return _CACHED_NC


def _enable_ldw_opt():
    """The default axon compile flags pass --enable-ldw-opt=false, which
    serializes every LDWEIGHTS with its MATMUL (~+107ns per matmul). Our
    kernel is a long stream of ldweights+matmul pairs, so re-enable it."""
    try:
        from concourse.compiler_utils import (get_compiler_flags,
                                              set_compiler_flags)
        flags = get_compiler_flags()
        new = [f.replace("--enable-ldw-opt=false", "--enable-ldw-opt=true")
               for f in flags]
        if new != flags:
            set_compiler_flags(new)
    except Exception:
        pass


def kernel(x, edge_index, W1, b1, W2, b2, _profile=False):
    g = FULL
    _enable_ldw_opt()
    in_maps = host_prep(g, x, edge_index, W1, b1, W2, b2)
    nc = _get_nc()
    res = run_bass_kernel_spmd(nc, in_maps, list(range(NCORES)),
                               trace=_profile)
    out = np.concatenate([res.results[c]["out"] for c in range(NCORES)], 0)
    out = out[:g.n].astype(np.float32)
    if _profile:
        kernel.last_result = res
    return out
